# revision 18
# baseline (speedup 1.0000x reference)
"""Trainium2 Bass kernel for nn_ClassificationHead.

reference semantics (B=8, S=4096, H=1024, L=128):
    seq_len = attention_mask.sum(-1)                       # per sample
    valid[s] = s < seq_len - 2                             # s in [0, S-2)
    context  = x[:, 1:-1, :] * valid                       # (B, S-2, H)
    logits   = context @ W.T + b                           # (B, S-2, L)
    return logits, context

Sharding: pure data parallel — one batch sample per NeuronCore (8 cores),
W/b replicated. The bass program is input-independent (mask is processed
on device), so one compiled SPMD program serves every call.
"""

import numpy as np
from contextlib import ExitStack

import concourse.bass as bass
import concourse.bacc as bacc
import concourse.mybir as mybir
import concourse.tile as tile
from concourse.bass_utils import run_bass_kernel_spmd

B, S, H, L = 8, 4096, 1024, 128
SOUT = S - 2                    # 4094 output tokens
N_CORES = 8
F32 = mybir.dt.float32
F32R = mybir.dt.float32r
I32 = mybir.dt.int32

HC = H // 128                   # 8 hidden chunks of 128
NT = (SOUT + 127) // 128        # 32 token tiles (last has 126 rows)
NG = NT // 4                    # 8 super-tiles of 512 tokens


def _emit(tc, nc, x_d, m_d, w_d, b_d, ident_d, lg_d, cx_d):
    ctx = ExitStack()
    const = ctx.enter_context(tc.tile_pool(name="const", bufs=1))
    xm_pool = ctx.enter_context(tc.tile_pool(name="xm", bufs=10))
    xt_pool = ctx.enter_context(tc.tile_pool(name="xt", bufs=10))
    lt_pool = ctx.enter_context(tc.tile_pool(name="lt", bufs=3))
    lg_pool = ctx.enter_context(tc.tile_pool(name="lg", bufs=3))
    ps_big = ctx.enter_context(tc.tile_pool(name="psb", bufs=4, space="PSUM"))
    ps_acc = ctx.enter_context(tc.tile_pool(name="psa", bufs=2, space="PSUM"))
    ps_sm = ctx.enter_context(tc.tile_pool(name="pss", bufs=2, space="PSUM"))

    with ctx:
        # ---------------- preamble: constants ----------------
        ident = const.tile([128, 128], F32)
        nc.sync.dma_start(ident[:], ident_d[:])

        ones_row = const.tile([1, 512], F32)
        nc.gpsimd.memset(ones_row[0:1, :], 1.0)
        ones_col = const.tile([128, 1], F32)
        nc.gpsimd.memset(ones_col[:, 0:1], 1.0)

        b_col = const.tile([128, 1], F32)
        nc.sync.dma_start(b_col[:, 0:1], b_d[:, None])

        # ---------------- mask -> valid[128, 32] ----------------
        mask_sb = const.tile([128, S // 128], I32)
        nc.sync.dma_start(mask_sb[:], m_d.rearrange("(p f) -> p f", p=128))
        maskf = const.tile([128, S // 128], F32)
        nc.vector.tensor_copy(maskf[:], mask_sb[:])
        partial = const.tile([128, 1], F32)
        nc.vector.reduce_sum(partial[:], maskf[:], axis=mybir.AxisListType.X)

        ps_seq = ps_sm.tile([1, 1], F32, tag="sm")
        nc.tensor.matmul(ps_seq[0:1, 0:1], partial[:, 0:1], ones_col[:, 0:1])
        thresh = const.tile([1, 1], F32)
        nc.vector.tensor_scalar_add(thresh[0:1, 0:1], ps_seq[0:1, 0:1], -2.0)

        ps_tb = ps_sm.tile([128, 1], F32, tag="sm")
        nc.tensor.matmul(ps_tb[:, 0:1], ones_row[0:1, 0:128], thresh[0:1, 0:1])
        threshb = const.tile([128, 1], F32)
        nc.vector.tensor_copy(threshb[:], ps_tb[:])

        iota_i = const.tile([128, NT], I32)
        nc.gpsimd.iota(iota_i[:], pattern=[[128, NT]], base=0, channel_multiplier=1)
        iota_f = const.tile([128, NT], F32)
        nc.vector.tensor_copy(iota_f[:], iota_i[:])
        validf = const.tile([128, NT], F32)
        nc.vector.tensor_scalar(
            validf[:], iota_f[:], threshb[:, 0:1], None, op0=mybir.AluOpType.is_lt
        )

        # ---------------- W -> WT chunks ----------------
        w_sb = const.tile([128, H], F32)
        nc.sync.dma_start(w_sb[:], w_d[:])
        wt_sb = const.tile([128, H], F32R)     # [hh, c*128 + l], rounded for fp32r matmul
        for c in range(HC):
            pw = ps_sm.tile([128, 128], F32, tag="sm")
            nc.tensor.transpose(pw[:], w_sb[:, c * 128:(c + 1) * 128], ident[:])
            nc.vector.tensor_copy(wt_sb[:, c * 128:(c + 1) * 128], pw[:])

        # ---------------- main loop over 512-token super-tiles ----------------
        # Software-pipelined: super-tile g's transposes are interleaved with
        # super-tile (g-1)'s accumulation matmuls so real matmuls pepper the
        # PE stream and keep the HAM clock-gate at K=8/8.
        def emit_loads(g):
            xm_tiles = []
            for k in range(4):
                t = 4 * g + k
                rows = min(128, SOUT - 128 * t)
                xm = xm_pool.tile([128, H], F32)
                if rows < 128:
                    nc.gpsimd.memset(xm[:], 0.0)
                nc.sync.dma_start(xm[:rows, :], x_d[1 + 128 * t: 1 + 128 * t + rows, :])
                nc.vector.tensor_scalar_mul(xm[:], xm[:], validf[:, t:t + 1])
                nc.gpsimd.dma_start(cx_d[128 * t: 128 * t + rows, :], xm[:rows, :])
                xm_tiles.append(xm)
            return xm_tiles

        def emit_store(g, acc):
            lt = lt_pool.tile([128, 512], F32)
            nc.scalar.activation(
                lt[:], acc[:], mybir.ActivationFunctionType.Identity, bias=b_col[:, 0:1]
            )
            lgbig = lg_pool.tile([128, 4, 128], F32)
            for k in range(4):
                plg = ps_sm.tile([128, 128], F32, tag="sm")
                nc.tensor.transpose(plg[:], lt[:, 128 * k:128 * (k + 1)], ident[:])
                nc.scalar.copy(lgbig[:, k, :], plg[:])
            if g < NG - 1:
                nc.sync.dma_start(
                    lg_d[512 * g: 512 * (g + 1), :].rearrange("(k p) l -> p k l", p=128),
                    lgbig[:],
                )
            else:
                nc.sync.dma_start(
                    lg_d[512 * g: 512 * g + 384, :].rearrange("(k p) l -> p k l", p=128),
                    lgbig[:, 0:3, :],
                )
                nc.sync.dma_start(lg_d[512 * g + 384: SOUT, :], lgbig[:126, 3, :])

        for g in range(NG):
            xm_tiles = emit_loads(g)
            acc = ps_acc.tile([128, 512], F32, name="acc", tag="acc")
            for c in range(HC):
                pxt = ps_big.tile([128, 512], F32)
                for k in range(4):
                    nc.tensor.transpose(
                        pxt[:, 128 * k:128 * (k + 1)],
                        xm_tiles[k][:, 128 * c:128 * (c + 1)],
                        ident[:],
                    )
                xt = xt_pool.tile([128, 512], F32R)
                if c % 2 == 0:
                    nc.vector.tensor_copy(xt[:], pxt[:])
                else:
                    nc.scalar.copy(xt[:], pxt[:])
                nc.tensor.matmul(
                    acc[:],
                    wt_sb[:, c * 128:(c + 1) * 128],
                    xt[:],
                    start=(c == 0),
                    stop=(c == HC - 1),
                )
            emit_store(g, acc)


def _build_nc():
    nc = bacc.Bacc()
    x_d = nc.dram_tensor("x", [S, H], F32, kind="ExternalInput")
    m_d = nc.dram_tensor("mask", [S], I32, kind="ExternalInput")
    w_d = nc.dram_tensor("w", [L, H], F32, kind="ExternalInput")
    b_d = nc.dram_tensor("b", [L], F32, kind="ExternalInput")
    ident_d = nc.dram_tensor("ident", [128, 128], F32, kind="ExternalInput")
    lg_d = nc.dram_tensor("logits", [SOUT, L], F32, kind="ExternalOutput")
    cx_d = nc.dram_tensor("context", [SOUT, H], F32, kind="ExternalOutput")
    with tile.TileContext(nc) as tc:
        _emit(tc, nc, x_d, m_d, w_d, b_d, ident_d, lg_d, cx_d)
    nc.compile()
    return nc


_cache = {}
last_result = None


def kernel(encoder_last_hidden_state, attention_mask, W, b, trace=False):
    global last_result
    x = np.ascontiguousarray(encoder_last_hidden_state, dtype=np.float32)
    mask = np.ascontiguousarray(attention_mask, dtype=np.int32)
    Wf = np.ascontiguousarray(W, dtype=np.float32)
    bf = np.ascontiguousarray(b, dtype=np.float32)
    ident = np.eye(128, dtype=np.float32)

    if "nc" not in _cache:
        _cache["nc"] = _build_nc()
    nc = _cache["nc"]

    in_maps = [
        {"x": x[i], "mask": mask[i], "w": Wf, "b": bf, "ident": ident}
        for i in range(N_CORES)
    ]
    res = run_bass_kernel_spmd(nc, in_maps, list(range(N_CORES)), trace=trace)
    last_result = res

    logits = np.stack([res.results[i]["logits"] for i in range(N_CORES)])
    context = np.stack([res.results[i]["context"] for i in range(N_CORES)])
    return logits, context


# revision 19
# speedup vs baseline: 1.0222x; 1.0222x over previous
"""Trainium2 Bass kernel for nn_ClassificationHead.

reference semantics (B=8, S=4096, H=1024, L=128):
    seq_len = attention_mask.sum(-1)                       # per sample
    valid[s] = s < seq_len - 2                             # s in [0, S-2)
    context  = x[:, 1:-1, :] * valid                       # (B, S-2, H)
    logits   = context @ W.T + b                           # (B, S-2, L)
    return logits, context

Sharding: pure data parallel — one batch sample per NeuronCore (8 cores),
W/b replicated. The bass program is input-independent (mask is processed
on device), so one compiled SPMD program serves every call.
"""

import numpy as np
from contextlib import ExitStack

import concourse.bass as bass
import concourse.bacc as bacc
import concourse.mybir as mybir
import concourse.tile as tile
from concourse.bass_utils import run_bass_kernel_spmd

B, S, H, L = 8, 4096, 1024, 128
SOUT = S - 2                    # 4094 output tokens
N_CORES = 8
F32 = mybir.dt.float32
F32R = mybir.dt.float32r
I32 = mybir.dt.int32

HC = H // 128                   # 8 hidden chunks of 128
NT = (SOUT + 127) // 128        # 32 token tiles (last has 126 rows)
NG = NT // 4                    # 8 super-tiles of 512 tokens


def _emit(tc, nc, x_d, m_d, w_d, b_d, ident_d, lg_d, cx_d):
    ctx = ExitStack()
    const = ctx.enter_context(tc.tile_pool(name="const", bufs=1))
    xm_pool = ctx.enter_context(tc.tile_pool(name="xm", bufs=10))
    xt_pool = ctx.enter_context(tc.tile_pool(name="xt", bufs=10))
    lt_pool = ctx.enter_context(tc.tile_pool(name="lt", bufs=3))
    lg_pool = ctx.enter_context(tc.tile_pool(name="lg", bufs=3))
    ps_big = ctx.enter_context(tc.tile_pool(name="psb", bufs=4, space="PSUM"))
    ps_acc = ctx.enter_context(tc.tile_pool(name="psa", bufs=2, space="PSUM"))
    ps_sm = ctx.enter_context(tc.tile_pool(name="pss", bufs=2, space="PSUM"))

    with ctx:
        # ---------------- preamble: constants ----------------
        ident = const.tile([128, 128], F32)
        nc.sync.dma_start(ident[:], ident_d[:])

        ones_row = const.tile([1, 512], F32)
        nc.gpsimd.memset(ones_row[0:1, :], 1.0)
        ones_col = const.tile([128, 1], F32)
        nc.gpsimd.memset(ones_col[:, 0:1], 1.0)

        b_col = const.tile([128, 1], F32)
        nc.sync.dma_start(b_col[:, 0:1], b_d[:, None])

        # ---------------- mask -> valid[128, 32] ----------------
        mask_sb = const.tile([128, S // 128], I32)
        nc.sync.dma_start(mask_sb[:], m_d.rearrange("(p f) -> p f", p=128))
        maskf = const.tile([128, S // 128], F32)
        nc.vector.tensor_copy(maskf[:], mask_sb[:])
        partial = const.tile([128, 1], F32)
        nc.vector.reduce_sum(partial[:], maskf[:], axis=mybir.AxisListType.X)

        ps_seq = ps_sm.tile([1, 1], F32, tag="sm")
        nc.tensor.matmul(ps_seq[0:1, 0:1], partial[:, 0:1], ones_col[:, 0:1])
        thresh = const.tile([1, 1], F32)
        nc.vector.tensor_scalar_add(thresh[0:1, 0:1], ps_seq[0:1, 0:1], -2.0)

        ps_tb = ps_sm.tile([128, 1], F32, tag="sm")
        nc.tensor.matmul(ps_tb[:, 0:1], ones_row[0:1, 0:128], thresh[0:1, 0:1])
        threshb = const.tile([128, 1], F32)
        nc.vector.tensor_copy(threshb[:], ps_tb[:])

        iota_i = const.tile([128, NT], I32)
        nc.gpsimd.iota(iota_i[:], pattern=[[128, NT]], base=0, channel_multiplier=1)
        iota_f = const.tile([128, NT], F32)
        nc.vector.tensor_copy(iota_f[:], iota_i[:])
        validf = const.tile([128, NT], F32)
        nc.vector.tensor_scalar(
            validf[:], iota_f[:], threshb[:, 0:1], None, op0=mybir.AluOpType.is_lt
        )

        # ---------------- W -> WT chunks ----------------
        w_sb = const.tile([128, H], F32)
        nc.sync.dma_start(w_sb[:], w_d[:])
        wt_sb = const.tile([128, H], F32R)     # [hh, c*128 + l], rounded for fp32r matmul
        for c in range(HC):
            pw = ps_sm.tile([128, 128], F32, tag="sm")
            nc.tensor.transpose(pw[:], w_sb[:, c * 128:(c + 1) * 128], ident[:])
            nc.vector.tensor_copy(wt_sb[:, c * 128:(c + 1) * 128], pw[:])

        # ---------------- main loop over 512-token super-tiles ----------------
        # Software-pipelined: super-tile g's transposes are interleaved with
        # super-tile (g-1)'s accumulation matmuls so real matmuls pepper the
        # PE stream and keep the HAM clock-gate at K=8/8.
        def emit_loads(g):
            xm_tiles = []
            for k in range(4):
                t = 4 * g + k
                rows = min(128, SOUT - 128 * t)
                xm = xm_pool.tile([128, H], F32)
                if rows < 128:
                    nc.gpsimd.memset(xm[:], 0.0)
                nc.sync.dma_start(xm[:rows, :], x_d[1 + 128 * t: 1 + 128 * t + rows, :])
                nc.vector.tensor_scalar_mul(xm[:], xm[:], validf[:, t:t + 1])
                nc.gpsimd.dma_start(cx_d[128 * t: 128 * t + rows, :], xm[:rows, :])
                xm_tiles.append(xm)
            return xm_tiles

        def emit_store(g, acc):
            lt = lt_pool.tile([128, 512], F32)
            nc.scalar.activation(
                lt[:], acc[:], mybir.ActivationFunctionType.Identity, bias=b_col[:, 0:1]
            )
            lgbig = lg_pool.tile([128, 4, 128], F32)
            for k in range(4):
                plg = ps_sm.tile([128, 128], F32, tag="sm")
                nc.tensor.transpose(plg[:], lt[:, 128 * k:128 * (k + 1)], ident[:])
                nc.scalar.copy(lgbig[:, k, :], plg[:])
            if g < NG - 1:
                nc.sync.dma_start(
                    lg_d[512 * g: 512 * (g + 1), :].rearrange("(k p) l -> p k l", p=128),
                    lgbig[:],
                )
            else:
                nc.sync.dma_start(
                    lg_d[512 * g: 512 * g + 384, :].rearrange("(k p) l -> p k l", p=128),
                    lgbig[:, 0:3, :],
                )
                nc.sync.dma_start(lg_d[512 * g + 384: SOUT, :], lgbig[:126, 3, :])

        for g in range(NG):
            xm_tiles = emit_loads(g)
            acc = ps_acc.tile([128, 512], F32, name="acc", tag="acc")
            for c in range(HC):
                pxt = ps_big.tile([128, 512], F32)
                for k in range(4):
                    nc.tensor.transpose(
                        pxt[:, 128 * k:128 * (k + 1)],
                        xm_tiles[k][:, 128 * c:128 * (c + 1)],
                        ident[:],
                    )
                xt = xt_pool.tile([128, 512], F32R)
                nc.vector.tensor_copy(xt[:], pxt[:])
                nc.tensor.matmul(
                    acc[:],
                    wt_sb[:, c * 128:(c + 1) * 128],
                    xt[:],
                    start=(c == 0),
                    stop=(c == HC - 1),
                )
            emit_store(g, acc)


def _build_nc():
    nc = bacc.Bacc()
    x_d = nc.dram_tensor("x", [S, H], F32, kind="ExternalInput")
    m_d = nc.dram_tensor("mask", [S], I32, kind="ExternalInput")
    w_d = nc.dram_tensor("w", [L, H], F32, kind="ExternalInput")
    b_d = nc.dram_tensor("b", [L], F32, kind="ExternalInput")
    ident_d = nc.dram_tensor("ident", [128, 128], F32, kind="ExternalInput")
    lg_d = nc.dram_tensor("logits", [SOUT, L], F32, kind="ExternalOutput")
    cx_d = nc.dram_tensor("context", [SOUT, H], F32, kind="ExternalOutput")
    with tile.TileContext(nc) as tc:
        _emit(tc, nc, x_d, m_d, w_d, b_d, ident_d, lg_d, cx_d)
    nc.compile()
    return nc


_cache = {}
last_result = None


def kernel(encoder_last_hidden_state, attention_mask, W, b, trace=False):
    global last_result
    x = np.ascontiguousarray(encoder_last_hidden_state, dtype=np.float32)
    mask = np.ascontiguousarray(attention_mask, dtype=np.int32)
    Wf = np.ascontiguousarray(W, dtype=np.float32)
    bf = np.ascontiguousarray(b, dtype=np.float32)
    ident = np.eye(128, dtype=np.float32)

    if "nc" not in _cache:
        _cache["nc"] = _build_nc()
    nc = _cache["nc"]

    in_maps = [
        {"x": x[i], "mask": mask[i], "w": Wf, "b": bf, "ident": ident}
        for i in range(N_CORES)
    ]
    res = run_bass_kernel_spmd(nc, in_maps, list(range(N_CORES)), trace=trace)
    last_result = res

    logits = np.stack([res.results[i]["logits"] for i in range(N_CORES)])
    context = np.stack([res.results[i]["context"] for i in range(N_CORES)])
    return logits, context


# revision 20
# speedup vs baseline: 1.0340x; 1.0116x over previous
"""Trainium2 Bass kernel for nn_ClassificationHead.

reference semantics (B=8, S=4096, H=1024, L=128):
    seq_len = attention_mask.sum(-1)                       # per sample
    valid[s] = s < seq_len - 2                             # s in [0, S-2)
    context  = x[:, 1:-1, :] * valid                       # (B, S-2, H)
    logits   = context @ W.T + b                           # (B, S-2, L)
    return logits, context

Sharding: pure data parallel — one batch sample per NeuronCore (8 cores),
W/b replicated. The bass program is input-independent (mask is processed
on device), so one compiled SPMD program serves every call.
"""

import numpy as np
from contextlib import ExitStack

import concourse.bass as bass
import concourse.bacc as bacc
import concourse.mybir as mybir
import concourse.tile as tile
from concourse.bass_utils import run_bass_kernel_spmd

B, S, H, L = 8, 4096, 1024, 128
SOUT = S - 2                    # 4094 output tokens
N_CORES = 8
F32 = mybir.dt.float32
F32R = mybir.dt.float32r
I32 = mybir.dt.int32

HC = H // 128                   # 8 hidden chunks of 128
NT = (SOUT + 127) // 128        # 32 token tiles (last has 126 rows)
NG = NT // 4                    # 8 super-tiles of 512 tokens


def _emit(tc, nc, x_d, m_d, w_d, b_d, ident_d, lg_d, cx_d):
    ctx = ExitStack()
    const = ctx.enter_context(tc.tile_pool(name="const", bufs=1))
    xm_pool = ctx.enter_context(tc.tile_pool(name="xm", bufs=10))
    xt_pool = ctx.enter_context(tc.tile_pool(name="xt", bufs=10))
    lt_pool = ctx.enter_context(tc.tile_pool(name="lt", bufs=3))
    lg_pool = ctx.enter_context(tc.tile_pool(name="lg", bufs=3))
    ps_big = ctx.enter_context(tc.tile_pool(name="psb", bufs=5, space="PSUM"))
    ps_acc = ctx.enter_context(tc.tile_pool(name="psa", bufs=2, space="PSUM"))
    ps_sm = ctx.enter_context(tc.tile_pool(name="pss", bufs=1, space="PSUM"))

    with ctx:
        # ---------------- preamble: constants ----------------
        ident = const.tile([128, 128], F32)
        nc.sync.dma_start(ident[:], ident_d[:])

        ones_row = const.tile([1, 512], F32)
        nc.gpsimd.memset(ones_row[0:1, :], 1.0)
        ones_col = const.tile([128, 1], F32)
        nc.gpsimd.memset(ones_col[:, 0:1], 1.0)

        b_col = const.tile([128, 1], F32)
        nc.sync.dma_start(b_col[:, 0:1], b_d[:, None])

        # ---------------- mask -> valid[128, 32] ----------------
        mask_sb = const.tile([128, S // 128], I32)
        nc.sync.dma_start(mask_sb[:], m_d.rearrange("(p f) -> p f", p=128))
        maskf = const.tile([128, S // 128], F32)
        nc.vector.tensor_copy(maskf[:], mask_sb[:])
        partial = const.tile([128, 1], F32)
        nc.vector.reduce_sum(partial[:], maskf[:], axis=mybir.AxisListType.X)

        ps_seq = ps_sm.tile([1, 1], F32, tag="sm")
        nc.tensor.matmul(ps_seq[0:1, 0:1], partial[:, 0:1], ones_col[:, 0:1])
        thresh = const.tile([1, 1], F32)
        nc.vector.tensor_scalar_add(thresh[0:1, 0:1], ps_seq[0:1, 0:1], -2.0)

        ps_tb = ps_sm.tile([128, 1], F32, tag="sm")
        nc.tensor.matmul(ps_tb[:, 0:1], ones_row[0:1, 0:128], thresh[0:1, 0:1])
        threshb = const.tile([128, 1], F32)
        nc.vector.tensor_copy(threshb[:], ps_tb[:])

        iota_i = const.tile([128, NT], I32)
        nc.gpsimd.iota(iota_i[:], pattern=[[128, NT]], base=0, channel_multiplier=1)
        iota_f = const.tile([128, NT], F32)
        nc.vector.tensor_copy(iota_f[:], iota_i[:])
        validf = const.tile([128, NT], F32)
        nc.vector.tensor_scalar(
            validf[:], iota_f[:], threshb[:, 0:1], None, op0=mybir.AluOpType.is_lt
        )

        # ---------------- W -> WT chunks ----------------
        w_sb = const.tile([128, H], F32)
        nc.sync.dma_start(w_sb[:], w_d[:])
        wt_sb = const.tile([128, H], F32R)     # [hh, c*128 + l], rounded for fp32r matmul
        for c in range(HC):
            pw = ps_sm.tile([128, 128], F32, tag="sm")
            nc.tensor.transpose(pw[:], w_sb[:, c * 128:(c + 1) * 128], ident[:])
            nc.vector.tensor_copy(wt_sb[:, c * 128:(c + 1) * 128], pw[:])

        # ---------------- main loop over 512-token super-tiles ----------------
        # Software-pipelined: super-tile g's transposes are interleaved with
        # super-tile (g-1)'s accumulation matmuls so real matmuls pepper the
        # PE stream and keep the HAM clock-gate at K=8/8.
        def emit_loads(g):
            xm_tiles = []
            for k in range(4):
                t = 4 * g + k
                rows = min(128, SOUT - 128 * t)
                xm = xm_pool.tile([128, H], F32)
                if rows < 128:
                    nc.gpsimd.memset(xm[:], 0.0)
                nc.sync.dma_start(xm[:rows, :], x_d[1 + 128 * t: 1 + 128 * t + rows, :])
                nc.vector.tensor_scalar_mul(xm[:], xm[:], validf[:, t:t + 1])
                nc.gpsimd.dma_start(cx_d[128 * t: 128 * t + rows, :], xm[:rows, :])
                xm_tiles.append(xm)
            return xm_tiles

        def emit_store(g, acc):
            lt = lt_pool.tile([128, 512], F32)
            nc.scalar.activation(
                lt[:], acc[:], mybir.ActivationFunctionType.Identity, bias=b_col[:, 0:1]
            )
            lgbig = lg_pool.tile([128, 4, 128], F32)
            for k in range(4):
                plg = ps_sm.tile([128, 128], F32, tag="sm")
                nc.tensor.transpose(plg[:], lt[:, 128 * k:128 * (k + 1)], ident[:])
                nc.scalar.copy(lgbig[:, k, :], plg[:])
            if g < NG - 1:
                nc.sync.dma_start(
                    lg_d[512 * g: 512 * (g + 1), :].rearrange("(k p) l -> p k l", p=128),
                    lgbig[:],
                )
            else:
                nc.sync.dma_start(
                    lg_d[512 * g: 512 * g + 384, :].rearrange("(k p) l -> p k l", p=128),
                    lgbig[:, 0:3, :],
                )
                nc.sync.dma_start(lg_d[512 * g + 384: SOUT, :], lgbig[:126, 3, :])

        for g in range(NG):
            xm_tiles = emit_loads(g)
            acc = ps_acc.tile([128, 512], F32, name="acc", tag="acc")
            for c in range(HC):
                pxt = ps_big.tile([128, 512], F32)
                for k in range(4):
                    nc.tensor.transpose(
                        pxt[:, 128 * k:128 * (k + 1)],
                        xm_tiles[k][:, 128 * c:128 * (c + 1)],
                        ident[:],
                    )
                xt = xt_pool.tile([128, 512], F32R)
                nc.vector.tensor_copy(xt[:], pxt[:])
                nc.tensor.matmul(
                    acc[:],
                    wt_sb[:, c * 128:(c + 1) * 128],
                    xt[:],
                    start=(c == 0),
                    stop=(c == HC - 1),
                )
            emit_store(g, acc)


def _build_nc():
    nc = bacc.Bacc()
    x_d = nc.dram_tensor("x", [S, H], F32, kind="ExternalInput")
    m_d = nc.dram_tensor("mask", [S], I32, kind="ExternalInput")
    w_d = nc.dram_tensor("w", [L, H], F32, kind="ExternalInput")
    b_d = nc.dram_tensor("b", [L], F32, kind="ExternalInput")
    ident_d = nc.dram_tensor("ident", [128, 128], F32, kind="ExternalInput")
    lg_d = nc.dram_tensor("logits", [SOUT, L], F32, kind="ExternalOutput")
    cx_d = nc.dram_tensor("context", [SOUT, H], F32, kind="ExternalOutput")
    with tile.TileContext(nc) as tc:
        _emit(tc, nc, x_d, m_d, w_d, b_d, ident_d, lg_d, cx_d)
    nc.compile()
    return nc


_cache = {}
last_result = None


def kernel(encoder_last_hidden_state, attention_mask, W, b, trace=False):
    global last_result
    x = np.ascontiguousarray(encoder_last_hidden_state, dtype=np.float32)
    mask = np.ascontiguousarray(attention_mask, dtype=np.int32)
    Wf = np.ascontiguousarray(W, dtype=np.float32)
    bf = np.ascontiguousarray(b, dtype=np.float32)
    ident = np.eye(128, dtype=np.float32)

    if "nc" not in _cache:
        _cache["nc"] = _build_nc()
    nc = _cache["nc"]

    in_maps = [
        {"x": x[i], "mask": mask[i], "w": Wf, "b": bf, "ident": ident}
        for i in range(N_CORES)
    ]
    res = run_bass_kernel_spmd(nc, in_maps, list(range(N_CORES)), trace=trace)
    last_result = res

    logits = np.stack([res.results[i]["logits"] for i in range(N_CORES)])
    context = np.stack([res.results[i]["context"] for i in range(N_CORES)])
    return logits, context
